# revision 14
# baseline (speedup 1.0000x reference)
"""Trainium2 Bass kernel for nn_Attention_30760555774660 (stacked attention VQA net).

Sharding: data-parallel over batch, 256 -> 8 cores x 32. Weights replicated.

Per-core plan (B=32 local batch, S=196, D=1024, A=512, O=3000):
  - img_b [196,1024] DMA'd once, PE-transposed (identity matmuls) into
    imgT_b [128, 8, 196] (d-on-partitions) for the two projections.
  - Projections img_b @ W_ia{1,2} run as float32r (full-rate fp32) matmuls,
    N=512, accumulating 8 K-chunks in PSUM.
  - The broadcast add of the q-projection row is folded into the same PSUM
    accumulation with a one-hot selector matmul (K=32).
  - tanh on ScalarE; logits via DVE tensor_tensor_reduce against
    partition-broadcast Wp; softmax batched per group of 4 batch elems
    ([4,196] rows after a PE transpose of the logit columns).
  - vI = pi @ img via matmul with pi column stationary, natural img moving.
  - u1/u2 kept transposed (u1T/u2T [128, 8, 32]) so the q-projection of
    block 2 and the final FC need no extra transposes.
  - Final FC streams W_fc [128,500] tiles against stationary u2T.
"""

import os
import sys

import numpy as np

if "/opt/trn_rl_repo" not in sys.path:
    sys.path.insert(0, "/opt/trn_rl_repo")

B_FULL = 256
N_CORES = 8
B = B_FULL // N_CORES  # 32
S = 196
D = 1024
A = 512
O = 3000
G = 4  # softmax group size
NG = B // G
DC = D // 128  # 8 d-chunks
S_CHUNKS = [(0, 128), (128, 68)]
OC = 6
ON = O // OC  # 500

_nc_cache = None


def _build_nc():
    import concourse.bacc as bacc
    import concourse.tile as tile
    from concourse import mybir

    f32 = mybir.dt.float32
    f32r = mybir.dt.float32r
    Tanh = mybir.ActivationFunctionType.Tanh
    Exp = mybir.ActivationFunctionType.Exp
    mult = mybir.AluOpType.mult
    add = mybir.AluOpType.add

    nc = bacc.Bacc("TRN2", target_bir_lowering=False)

    img_h = nc.dram_tensor("img", [B, S, D], f32r, kind="ExternalInput")
    ques_h = nc.dram_tensor("ques", [B, D], f32, kind="ExternalInput")
    wia1_h = nc.dram_tensor("W_ia1", [D, A], f32r, kind="ExternalInput")
    wqa1_h = nc.dram_tensor("W_qa1", [D, A], f32r, kind="ExternalInput")
    bqa1_h = nc.dram_tensor("b_qa1", [A], f32, kind="ExternalInput")
    wp1_h = nc.dram_tensor("Wp1", [A], f32, kind="ExternalInput")
    wia2_h = nc.dram_tensor("W_ia2", [D, A], f32r, kind="ExternalInput")
    wqa2_h = nc.dram_tensor("W_qa2", [D, A], f32r, kind="ExternalInput")
    bqa2_h = nc.dram_tensor("b_qa2", [A], f32, kind="ExternalInput")
    wp2_h = nc.dram_tensor("Wp2", [A], f32, kind="ExternalInput")
    wfc_h = nc.dram_tensor("W_fc", [D, O], f32r, kind="ExternalInput")
    bfc_h = nc.dram_tensor("b_fc", [O], f32, kind="ExternalInput")
    oneh_h = nc.dram_tensor("ONEHOTS", [B, B, 128], f32r, kind="ExternalInput")
    ident_h = nc.dram_tensor("IDENT", [128, 128], f32r, kind="ExternalInput")
    identf_h = nc.dram_tensor("IDENTF", [128, 128], f32, kind="ExternalInput")
    score_h = nc.dram_tensor("score", [B, O], f32, kind="ExternalOutput")

    import bass_rust  # noqa: F401
    import concourse.bass as bass  # noqa: F401

    def bcast_ap(h, n_part, free_n):
        # partition-stride-0 broadcast read of a 1-D dram tensor
        ap = h[:]
        return bass.AP(tensor=ap.tensor, offset=ap.offset, ap=[[0, n_part]] + ap.ap)

    with tile.TileContext(nc) as tc:
        with (
            tc.tile_pool(name="const", bufs=1) as const,
            tc.tile_pool(name="imgn", bufs=5) as imgn_p,
            tc.tile_pool(name="imgt", bufs=3) as imgt_p,
            tc.tile_pool(name="p2sb", bufs=5) as p2sb_p,
            tc.tile_pool(name="work", bufs=2) as work,
            tc.tile_pool(name="work1", bufs=1) as work1,
            tc.tile_pool(name="wstream", bufs=3) as wstream,
            tc.tile_pool(name="pst", bufs=5, space="PSUM") as pst,
            tc.tile_pool(name="psp", bufs=3, space="PSUM") as psp,
        ):
            # ---------------- constants ----------------
            ident = const.tile([128, 128], f32r)
            nc.sync.dma_start(out=ident, in_=ident_h[:, :])
            identf = const.tile([128, 128], f32)
            nc.sync.dma_start(out=identf, in_=identf_h[:, :])
            oneh = const.tile([B, B, 128], f32r)
            nc.sync.dma_start(out=oneh, in_=oneh_h[:, :, :])
            wia1 = const.tile([128, DC, A], f32r)
            nc.sync.dma_start(out=wia1, in_=wia1_h[:, :].rearrange("(c p) a -> p c a", p=128))
            wia2 = const.tile([128, DC, A], f32r)
            nc.sync.dma_start(out=wia2, in_=wia2_h[:, :].rearrange("(c p) a -> p c a", p=128))
            wqa2 = const.tile([128, DC, A], f32r)
            nc.sync.dma_start(out=wqa2, in_=wqa2_h[:, :].rearrange("(c p) a -> p c a", p=128))
            bqa1b = const.tile([B, A], f32)
            nc.gpsimd.dma_start(out=bqa1b, in_=bcast_ap(bqa1_h, B, A))
            bqa2b = const.tile([B, A], f32)
            nc.gpsimd.dma_start(out=bqa2b, in_=bcast_ap(bqa2_h, B, A))
            wp1b = const.tile([128, A], f32)
            nc.gpsimd.dma_start(out=wp1b, in_=bcast_ap(wp1_h, 128, A))
            wp2b = const.tile([128, A], f32)
            nc.gpsimd.dma_start(out=wp2b, in_=bcast_ap(wp2_h, 128, A))
            quesA = const.tile([B, D], f32)
            nc.sync.dma_start(out=quesA, in_=ques_h[:, :])

            quesT = const.tile([128, DC, B], f32r)
            QP1 = const.tile([B, A], f32r)
            QP2 = const.tile([B, A], f32r)
            u1T = const.tile([128, DC, B], f32r)
            u2T = const.tile([128, DC, B], f32r)
            nc.vector.memset(QP2[:, :].bitcast(f32), 0.0)
            nc.vector.memset(u1T[:, :, :].bitcast(f32), 0.0)

            def r(ap):
                return ap

            # quesT[p, c, b] = ques[b, c*128+p]
            for c in range(DC):
                pt = pst.tile([128, B], f32, tag="tr")
                nc.tensor.transpose(pt, quesA[:, c * 128 : (c + 1) * 128], identf[0:B, 0:B])
                nc.vector.tensor_copy(quesT[:, c, :], pt)

            # QP1 = ques @ W_qa1 + b_qa1   [32, 512]
            qp_ps = psp.tile([B, A], f32, tag="pp")
            for c in range(DC):
                wq = wstream.tile([128, A], f32r, tag="ws")
                nc.sync.dma_start(out=wq, in_=wqa1_h[c * 128 : (c + 1) * 128, :])
                nc.tensor.matmul(qp_ps, r(quesT[:, c, :]), r(wq), start=(c == 0), stop=(c == DC - 1))
            nc.vector.tensor_add(QP1, qp_ps, bqa1b)

            imgN = {}
            imgT = {}
            p2sb = {}
            Lc1 = {}
            Lc2 = {}

            def load_and_proj(b):
                """DMA img_b, transpose, run both projections. Block-1 proj
                gets the QP1 broadcast folded in and goes through tanh+logits;
                block-2 proj parks in SBUF."""
                inb = imgn_p.tile([128, 2, D], f32r, tag="imgn")
                imgN[b] = inb
                nc.sync.dma_start(out=inb[:, 0, :], in_=img_h[b : b + 1, 0:128, :].rearrange("o s d -> (o s) d"))
                nc.sync.dma_start(out=inb[0:68, 1, :], in_=img_h[b : b + 1, 128:196, :].rearrange("o s d -> (o s) d"))
                itb = imgt_p.tile([128, DC, S], f32r, tag="imgt")
                imgT[b] = itb
                for c in range(DC):
                    pa = pst.tile([128, 128], f32, tag="tr")
                    nc.tensor.transpose(pa, inb[0:128, 0, c * 128 : (c + 1) * 128].bitcast(f32), identf)
                    nc.vector.tensor_copy(itb[:, c, 0:128], pa)
                    pb = pst.tile([128, 128], f32, tag="tr")
                    nc.tensor.transpose(pb[:, 0:68], inb[0:68, 1, c * 128 : (c + 1) * 128].bitcast(f32), identf[0:68, 0:68])
                    nc.vector.tensor_copy(itb[:, c, 128:196], pb[:, 0:68])
                # block-1 projection + QP1 broadcast + tanh + logits
                lc = work.tile([128, 2, G], f32, tag="lc1")
                if b % G == 0:
                    Lc1[b // G] = lc
                lc = Lc1[b // G]
                for si, (s0, sl) in enumerate(S_CHUNKS):
                    pp = psp.tile([128, A], f32, tag="pp")
                    for c in range(DC):
                        nc.tensor.matmul(
                            pp[0:sl, :], r(itb[0:128, c, s0 : s0 + sl]), r(wia1[:, c, :]),
                            start=(c == 0), stop=False,
                        )
                    nc.tensor.matmul(pp[0:sl, :], r(oneh[:, b, 0:sl]), r(QP1), start=False, stop=True)
                    ha = work.tile([128, A], f32, tag="ha")
                    nc.scalar.activation(ha[0:sl], pp[0:sl], Tanh)
                    prod = work.tile([128, A], f32, tag="prod")
                    nc.vector.tensor_mul(prod[0:sl], ha[0:sl], wp1b[0:sl])
                    nc.vector.tensor_reduce(
                        lc[0:sl, si, b % G : b % G + 1], prod[0:sl],
                        axis=mybir.AxisListType.X, op=add,
                    )
                # block-2 projection -> SBUF
                p2 = p2sb_p.tile([128, 2, A], f32, tag="p2sb")
                p2sb[b] = p2
                for si, (s0, sl) in enumerate(S_CHUNKS):
                    pp = psp.tile([128, A], f32, tag="pp")
                    for c in range(DC):
                        nc.tensor.matmul(
                            pp[0:sl, :], r(itb[0:128, c, s0 : s0 + sl]), r(wia2[:, c, :]),
                            start=(c == 0), stop=(c == DC - 1),
                        )
                    nc.vector.tensor_copy(p2[0:sl, si, :], pp[0:sl])

            def softmax_and_pi(lc, tagp):
                """logit columns [128, 2, G] -> PI [G, 196] softmax rows."""
                LT = work.tile([G, S], f32, tag="LT")
                pa = pst.tile([128, 128], f32, tag="tr")
                nc.tensor.transpose(pa[0:G, :], lc[:, 0, :], identf)
                nc.vector.tensor_copy(LT[:, 0:128], pa[0:G, :])
                pb = pst.tile([128, 128], f32, tag="tr")
                nc.tensor.transpose(pb[0:G, 0:68], lc[0:68, 1, :], identf[0:68, 0:68])
                nc.vector.tensor_copy(LT[:, 128:196], pb[0:G, 0:68])
                E = work.tile([G, S], f32, tag="E")
                Z = work.tile([G, 1], f32, tag="Z")
                nc.scalar.activation(E, LT, Exp, accum_out=Z)
                R = work.tile([G, 1], f32, tag="R")
                nc.vector.reciprocal(R, Z)
                PI = work.tile([G, S], f32, tag=tagp)
                nc.vector.tensor_scalar_mul(PI, E, R)
                return PI

            def weighted_sum_add(PI, g, other, out_tag, pool=None):
                """out[bb,:] = vI_bb + other[bb,:], via masked-column pi^T
                stationaries accumulating the whole group in one PSUM tile."""
                piTm = work.tile([128, 2, G, G], f32r, tag="piTm")
                for bb in range(G):
                    PIm = work.tile([G, S], f32, tag="PIm")
                    nc.vector.tensor_scalar_mul(PIm, PI, oneh[0:G, bb, 0:1].bitcast(f32))
                    pc = pst.tile([128, 128], f32, tag="tr")
                    nc.tensor.transpose(pc[:, 0:G], PIm[:, 0:128], identf[0:G, 0:G])
                    nc.vector.tensor_copy(piTm[:, 0, bb, :], pc[:, 0:G])
                    pd = pst.tile([128, 128], f32, tag="tr")
                    nc.tensor.transpose(pd[0:68, 0:G], PIm[:, 128:196], identf[0:G, 0:G])
                    nc.vector.tensor_copy(piTm[0:68, 1, bb, :], pd[0:68, 0:G])
                out = (pool or work).tile([G, D], f32, tag=out_tag)
                for h in range(2):
                    vp = psp.tile([G, A], f32, tag="pp")
                    k = 0
                    for bb in range(G):
                        inb = imgN[g * G + bb]
                        for si, (s0, sl) in enumerate(S_CHUNKS):
                            nc.tensor.matmul(
                                vp, r(piTm[0:sl, si, bb, :]), r(inb[0:sl, si, h * A : (h + 1) * A]),
                                start=(k == 0), stop=(k == 2 * G - 1),
                            )
                            k += 1
                    nc.vector.tensor_add(out[:, h * A : (h + 1) * A], vp, other[:, h * A : (h + 1) * A])
                return out

            NG_RUN = int(os.environ.get("NG_RUN", str(NG)))
            for g in range(NG_RUN):
                g4 = g * G
                for bb in range(G):
                    load_and_proj(g4 + bb)
                # ---- block 1 softmax / vI / u1 ----
                PI1 = softmax_and_pi(Lc1[g], "PI1")
                qn = work1.tile([G, D], f32, tag="qn")
                nc.sync.dma_start(out=qn, in_=ques_h[g4 : g4 + G, :])
                u1g = weighted_sum_add(PI1, g, qn, "u1g")
                for c in range(DC):
                    pt = pst.tile([128, 128], f32, tag="tr")
                    nc.tensor.transpose(pt[:, 0:G], u1g[:, c * 128 : (c + 1) * 128], identf[0:G, 0:G])
                    nc.vector.tensor_copy(u1T[:, c, g4 : g4 + G], pt[:, 0:G])
                # qp2 for this group (M=32, only rows g4:g4+G fresh)
                q2p = psp.tile([B, A], f32, tag="pp")
                for c in range(DC):
                    nc.tensor.matmul(q2p, r(u1T[:, c, :]), r(wqa2[:, c, :]), start=(c == 0), stop=(c == DC - 1))
                nc.vector.tensor_add(QP2, q2p, bqa2b)
                # ---- block 2 ----
                lc2 = work.tile([128, 2, G], f32, tag="lc2")
                Lc2[g] = lc2
                for bb in range(G):
                    b = g4 + bb
                    for si, (s0, sl) in enumerate(S_CHUNKS):
                        pp = psp.tile([128, A], f32, tag="pp")
                        nc.tensor.matmul(pp[0:sl, :], r(oneh[:, b, 0:sl]), r(QP2), start=True, stop=True)
                        ha2 = work.tile([128, A], f32, tag="ha")
                        nc.vector.tensor_add(ha2[0:sl], pp[0:sl], p2sb[b][0:sl, si, :])
                        nc.scalar.activation(ha2[0:sl], ha2[0:sl], Tanh)
                        prod = work.tile([128, A], f32, tag="prod")
                        nc.vector.tensor_mul(prod[0:sl], ha2[0:sl], wp2b[0:sl])
                        nc.vector.tensor_reduce(
                            lc2[0:sl, si, bb : bb + 1], prod[0:sl],
                            axis=mybir.AxisListType.X, op=add,
                        )
                PI2 = softmax_and_pi(lc2, "PI2")
                u2g = weighted_sum_add(PI2, g, u1g, "u2g", pool=work1)
                for c in range(DC):
                    pt = pst.tile([128, 128], f32, tag="tr")
                    nc.tensor.transpose(pt[:, 0:G], u2g[:, c * 128 : (c + 1) * 128], identf[0:G, 0:G])
                    nc.vector.tensor_copy(u2T[:, c, g4 : g4 + G], pt[:, 0:G])

            # ---------------- final FC ----------------
            for n in range(OC):
                fp = psp.tile([B, ON], f32, tag="pp")
                for c in range(DC):
                    wf = wstream.tile([128, ON], f32r, tag="ws")
                    nc.sync.dma_start(out=wf, in_=wfc_h[c * 128 : (c + 1) * 128, n * ON : (n + 1) * ON])
                    nc.tensor.matmul(fp, r(u2T[:, c, :]), r(wf), start=(c == 0), stop=(c == DC - 1))
                bf = work1.tile([B, ON], f32, tag="bf")
                nc.gpsimd.dma_start(
                    out=bf,
                    in_=_slice_bcast(bfc_h, B, n * ON, ON),
                )
                sc = work.tile([B, ON], f32, tag="sc")
                nc.vector.tensor_add(sc, fp, bf)
                nc.sync.dma_start(out=score_h[:, n * ON : (n + 1) * ON], in_=sc)

    nc.compile()
    return nc


def _slice_bcast(h, n_part, off, n):
    import concourse.bass as bass

    ap = h[off : off + n]
    return bass.AP(tensor=ap.tensor, offset=ap.offset, ap=[[0, n_part]] + ap.ap)


def _get_nc():
    global _nc_cache
    if _nc_cache is None:
        _nc_cache = _build_nc()
    return _nc_cache


def _make_in_maps(inputs):
    onehots = np.ascontiguousarray(
        np.repeat(np.eye(B, dtype=np.float32)[:, :, None], 128, axis=2)
    )
    ident = np.eye(128, dtype=np.float32)
    shared = {
        "W_ia1": np.ascontiguousarray(inputs["W_ia1"], np.float32),
        "W_qa1": np.ascontiguousarray(inputs["W_qa1"], np.float32),
        "b_qa1": np.ascontiguousarray(inputs["b_qa1"], np.float32),
        "Wp1": np.ascontiguousarray(inputs["Wp1"], np.float32),
        "W_ia2": np.ascontiguousarray(inputs["W_ia2"], np.float32),
        "W_qa2": np.ascontiguousarray(inputs["W_qa2"], np.float32),
        "b_qa2": np.ascontiguousarray(inputs["b_qa2"], np.float32),
        "Wp2": np.ascontiguousarray(inputs["Wp2"], np.float32),
        "W_fc": np.ascontiguousarray(inputs["W_fc"], np.float32),
        "b_fc": np.ascontiguousarray(inputs["b_fc"], np.float32),
        "ONEHOTS": onehots,
        "IDENT": ident,
        "IDENTF": ident,
    }
    in_maps = []
    for c in range(N_CORES):
        sl = slice(c * B, (c + 1) * B)
        m = dict(shared)
        m["img"] = np.ascontiguousarray(inputs["img_feat"][sl], np.float32)
        m["ques"] = np.ascontiguousarray(inputs["ques_feat"][sl], np.float32)
        in_maps.append(m)
    return in_maps


def kernel_run(inputs, trace=False):
    from concourse.bass_utils import run_bass_kernel_spmd

    nc = _get_nc()
    in_maps = _make_in_maps(inputs)
    res = run_bass_kernel_spmd(nc, in_maps, core_ids=list(range(N_CORES)), trace=trace)
    out = np.concatenate([r["score"] for r in res.results], axis=0)
    return out, res


def kernel(**inputs):
    out, _ = kernel_run(inputs)
    return out
